# revision 29
# baseline (speedup 1.0000x reference)
"""Trainium2 Bass kernel for nn_MultiHeadAttention_8400956031164.

Full attention block: QKV proj + per-head RMSNorm + RoPE + sliding-window
causal attention (WIN=1024) + output proj. All matmuls in bfloat16
(fp32 PSUM accumulation), ~3.5e-3 rel err vs the f32 reference.

Sharding: tensor-parallel over heads across 8 cores (2 heads/core), both
batches looped per core. Host sums the 8 partial (bf16) Wo outputs.

Per core, software-pipelined over the 8 flattened (batch, group) units:
iteration i runs inputs-DMA(i) / proj(i) / Wo(i-2) / attention(i-1), so
the norm/RoPE/transpose tail of proj(i) and the softmax-normalize tail of
attention(i-1) hide behind PE matmul phases.

  - QKV proj token-major: psqk [128 tok, q0|q1|k0|k1] accumulated over 16
    dk chunks (lhsT = X^T chunk shared with the V chain).
  - RMSNorm via ACT Square with accum_out per head block (no PE
    ones-matmul); rsqrt as exp(-0.5*ln(x)) on [128,4] (table-thrash-free).
  - RoPE token-major on DVE: shifted-view muls (rotate_half as access
    pattern; signs + q/k scales + softmax SCALE folded into host-packed
    cos/sin tables), rstd applied last (RoPE is linear, rstd per-token).
  - q^T/k^T for the scores matmuls via PE transposes into a half-bank
    bf16 PSUM tile + one strided DVE copy into the fused qkr tile.
  - Scores transposed S^T[k, q] per 128x(<=512) block over the sliding
    window; exp on ACT; causal/window triangles via gpsimd affine_select;
    softmax denominator via all-ones matmul accumulation; PV accumulates
    V^T @ P^T in PSUM; 1/L as exp(-ln(L)) on ACT (DVE reciprocal is slow).
  - Wo: lhsT = normalized out^T slices, 2-head PSUM accumulation,
    ACT/DVE copies, bf16 DMA out.
"""

import functools

import numpy as np
from contextlib import ExitStack

import concourse.bass as bass
import concourse.tile as tile
import concourse.mybir as mybir
import concourse.hw_specs as _hw_specs
from concourse import bacc, bass_utils


def _patch_act_tables():
    """Steer every activation in this kernel (Square/Ln/Exp/Copy) to the one
    ACT table set that really contains them all (natural_log_exp_and_others),
    so the greedy first-containing-set chooser never thrashes table loads.
    Only *removes* candidate sets; chosen ids still match act_info.json."""
    if getattr(_patch_act_tables, "done", False):
        return
    orig = _hw_specs.get_activation_tables
    AFT = mybir.ActivationFunctionType
    drop = {AFT.Exp, AFT.Ln, AFT.Square, AFT.Copy, AFT.Identity}

    @functools.cache
    def patched(module_arch):
        t = {k: set(v) for k, v in orig(module_arch).items()}
        for name, funcs in t.items():
            if name != "natural_log_exp_and_others":
                funcs -= drop
        return t

    _hw_specs.get_activation_tables = patched
    bacc.get_activation_tables = patched
    _patch_act_tables.done = True


_patch_act_tables()

B, S, D, H, HD, WIN = 2, 2048, 2048, 16, 128, 1024
EPS = 1e-6
SCALE = HD ** -0.5
NCORES = 8
HLOC = H // NCORES          # heads per core = 2
NL = HLOC * HD              # local head dims = 256
SG = 512                    # token group size
G = S // SG                 # groups per batch = 4
NDK = D // 128              # contraction chunks = 16

F32 = mybir.dt.float32
F32R = mybir.dt.float32r
BF16 = mybir.dt.bfloat16
AF = mybir.ActivationFunctionType

_CACHE = {}


def _build(repeat=1, mode="full"):
    # mode: "full" | "nodmain" (input DMAs hoisted out of the repeat loop,
    # timing-only, wrong results) | "dmaonly" (DMAs but no compute,
    # timing-only, wrong results)
    nc = bacc.Bacc(trn_type="TRN2", target_bir_lowering=False, debug=False)

    def din(name, shape, dt):
        return nc.dram_tensor(name, shape, dt, kind="ExternalInput").ap()

    # All inputs are host-pre-tiled to be partition-major contiguous so every
    # DMA is ~128 descriptors of large contiguous runs.
    xt = din("xt", [B * G, 128, NDK * SG], BF16)      # per (b,g) [128, 16*512]
    wqk = din("wqk", [128, NDK * 4 * HD], BF16)       # per dk: q0|q1|k0|k1
    wv = din("wv", [128, NDK * NL], BF16)
    wo = din("wo", [128, HLOC * D], BF16)
    # token-major cos/sin packs: per (b,g) [128 tok, (st, cos|sin), 512]
    cs = din("cs", [B * G, 128, 8 * SG], BF16)
    ones_d = din("ones_d", [128, 128], BF16)
    ident_d = din("ident_d", [128, 128], BF16)
    opart = nc.dram_tensor("opart", [B * S, D], BF16,
                           kind="ExternalOutput").ap()

    with tile.TileContext(nc) as tc, ExitStack() as ctx:
        const = ctx.enter_context(tc.tile_pool(name="const", bufs=1))
        wpool = ctx.enter_context(tc.tile_pool(name="w", bufs=1))
        xpool = ctx.enter_context(tc.tile_pool(name="x", bufs=3))
        cspool = ctx.enter_context(tc.tile_pool(name="cs", bufs=2))
        qpool = ctx.enter_context(tc.tile_pool(name="qr", bufs=5))
        vpool = ctx.enter_context(tc.tile_pool(name="v", bufs=18))
        rpool = ctx.enter_context(tc.tile_pool(name="rms", bufs=2))
        ppool = ctx.enter_context(tc.tile_pool(name="p", bufs=4))
        lpool = ctx.enter_context(tc.tile_pool(name="lin", bufs=2))
        opool = ctx.enter_context(tc.tile_pool(name="osb", bufs=6))
        outp = ctx.enter_context(tc.tile_pool(name="out", bufs=2))
        psA = ctx.enter_context(tc.tile_pool(name="psA", bufs=2, space="PSUM"))
        psT = ctx.enter_context(tc.tile_pool(name="psT", bufs=1, space="PSUM"))
        psS = ctx.enter_context(tc.tile_pool(name="psS", bufs=3, space="PSUM"))
        psO = ctx.enter_context(tc.tile_pool(name="psO", bufs=1, space="PSUM"))
        psL = ctx.enter_context(tc.tile_pool(name="psL", bufs=1, space="PSUM"))

        ones_t = const.tile([128, 128], BF16, tag="ones")
        nc.sync.dma_start(ones_t[:], ones_d)
        ident_t = const.tile([128, 128], BF16, tag="ident")
        nc.sync.dma_start(ident_t[:], ident_d)
        eps_t = const.tile([128, 1], F32, tag="eps")
        nc.vector.memset(eps_t[:], EPS)

        # Startup DMAs split across both hwdge queues (Sync + ACT) so the
        # weight stream and the first group's activation stream transfer in
        # parallel; the first qk-chain needs only wqk + xh0 + xh1.
        # wqk split in two so the first qk-chain (dk 0-7) starts after 1MB
        # instead of waiting for the whole 2MB weight tile
        wqk_t = wpool.tile([128, NDK, 4 * HD], BF16, tag="wqk")
        wqk_src = wqk.rearrange("p (a n) -> p a n", n=4 * HD)
        nc.scalar.dma_start(wqk_t[:, 0:8, :], wqk_src[:, 0:8, :])
        nc.scalar.dma_start(wqk_t[:, 8:16, :], wqk_src[:, 8:16, :])
        pre_xh0 = xpool.tile([128, 8, SG], BF16, tag="xt")
        nc.sync.dma_start(
            pre_xh0[:], xt[0].rearrange("p (a s) -> p a s", s=SG)[:, 0:8, :])
        pre_xh1 = xpool.tile([128, 8, SG], BF16, tag="xt")
        nc.sync.dma_start(
            pre_xh1[:], xt[0].rearrange("p (a s) -> p a s", s=SG)[:, 8:16, :])
        # first group's cos/sin split so the st0 slice lands before the st0
        # RoPE needs it (~26us) instead of behind the whole 1MB pack
        pre_cst = cspool.tile([128, 8, SG], BF16, tag="cs")
        cs0_src = cs[0].rearrange("p (f s) -> p f s", s=SG)
        nc.sync.dma_start(pre_cst[:, 0:2, :], cs0_src[:, 0:2, :])
        nc.sync.dma_start(pre_cst[:, 2:8, :], cs0_src[:, 2:8, :])
        # wv on the third (gpsimd) DMA queue so the first v-chain doesn't
        # wait behind both wqk halves on the scalar queue
        wv_t = wpool.tile([128, NDK, NL], BF16, tag="wv")
        nc.gpsimd.dma_start(wv_t[:], wv.rearrange("p (a n) -> p a n", n=NL))
        wo_t = wpool.tile([128, HLOC, D], BF16, tag="wo")
        nc.scalar.dma_start(wo_t[:], wo.rearrange("p (c d) -> p c d", d=D))

        hoist = {}
        dummy_out = None

        def emit_wo(osbs_, s0_):
            for st in range(4):
                ot = outp.tile([128, D], BF16, tag="out")
                for dg in range(4):
                    pso = psA.tile([128, 512], F32, tag="a")
                    nc.tensor.matmul(pso[:],
                                     osbs_[0][:, st * 128:(st + 1) * 128],
                                     wo_t[:, 0, dg * 512:(dg + 1) * 512],
                                     start=True, stop=False)
                    nc.tensor.matmul(pso[:],
                                     osbs_[1][:, st * 128:(st + 1) * 128],
                                     wo_t[:, 1, dg * 512:(dg + 1) * 512],
                                     start=False, stop=True)
                    dst = ot[:, dg * 512:(dg + 1) * 512]
                    if dg % 2 == 0:
                        nc.scalar.copy(dst, pso[:])
                    else:
                        nc.vector.tensor_copy(dst, pso[:])
                row = s0_ + st * 128
                nc.sync.dma_start(opart[row:row + 128, :], ot[:])

        pending_wo = []

        def emit_attention(b, g, qr_tiles, KrT, Vt):
            """Sliding-window attention for one (b, g) group, both heads."""
            osbs = {}
            for h in range(HLOC):
                qr_t = qr_tiles[h]
                kts = list(range(max(0, 4 * g - 8), 4 * g + 4))
                oacc = psO.tile([128, SG], F32, tag="o")
                lacc = psL.tile([128, SG], F32, tag="l")

                pend = []

                def emit_pv(item, first, last):
                    kt, qoff, n, p = item
                    nc.tensor.matmul(
                        oacc[:, qoff:qoff + n],
                        Vt[kt][:, h * HD:(h + 1) * HD], p[:],
                        start=first, stop=last)
                    nc.tensor.matmul(
                        lacc[:, qoff:qoff + n], ones_t[:], p[:],
                        start=first, stop=last)

                LAG = 3
                for i, kt in enumerate(kts):
                    qt_lo = max(4 * g, kt)
                    qt_hi = min(4 * g + 3, kt + 8)
                    qoff = 128 * (qt_lo - 4 * g)
                    n = 128 * (qt_hi - qt_lo + 1)
                    sc = psS.tile([128, n], F32, tag="score")
                    kr_t = KrT[(h, kt // 4)]
                    c = (kt % 4) * 128
                    nc.tensor.matmul(sc[:], kr_t[:, c:c + 128],
                                     qr_t[:, qoff:qoff + n],
                                     start=True, stop=True)
                    p = ppool.tile([128, n], BF16, tag="p")
                    nc.scalar.activation(p[:], sc[:], AF.Exp)
                    if kt >= 4 * g:
                        # causal triangle: keep kk <= qq (iota = qq-kk >= 0)
                        nc.gpsimd.affine_select(
                            p[:, 0:128], p[:, 0:128], pattern=[[1, 128]],
                            compare_op=mybir.AluOpType.is_ge, fill=0.0,
                            base=0, channel_multiplier=-1)
                    if kt + 8 <= 4 * g + 3:
                        # window edge: keep kk >= qq (iota = kk-qq >= 0)
                        nc.gpsimd.affine_select(
                            p[:, n - 128:n], p[:, n - 128:n],
                            pattern=[[-1, 128]],
                            compare_op=mybir.AluOpType.is_ge, fill=0.0,
                            base=0, channel_multiplier=1)
                    pend.append((kt, qoff, n, p))
                    if i >= LAG:
                        emit_pv(pend[i - LAG], first=(i - LAG == 0),
                                last=False)
                nk = len(kts)
                for j in range(max(0, nk - LAG), nk):
                    emit_pv(pend[j], first=(j == 0), last=(j == nk - 1))

                # 1/L = exp(-ln(L)) on ACT (same table set; DVE RECIPROCAL
                # is ~3.4us for [128,512] and sat on the critical path)
                linv = lpool.tile([128, SG], F32, tag="lin")
                nc.scalar.activation(linv[:], lacc[:], AF.Ln)
                nc.scalar.activation(linv[:], linv[:], AF.Exp, scale=-1.0)
                osb = opool.tile([128, SG], BF16, tag="osb")
                nc.vector.tensor_mul(osb[:], oacc[:], linv[:])
                osbs[h] = osb
            return osbs

        # Software pipeline over the flattened (b, g) sequence:
        #   iter i: inputs(i) DMA, proj(i) [PE], Wo(i-2) [PE],
        #           attention(i-1) [PE].
        # So the transpose-DMAs + RoPE tail of proj(i) hide behind
        # attention(i-1), and attention(i-1)'s normalize tail hides behind
        # proj(i+1) before Wo(i-1) consumes it.
        for rep in range(repeat):
          KrT = {}   # (b, h, g) -> [128, SG] bf16 K^T tile
          Vt = {}    # (b, s-tile) -> [128, NL] bf16 tile
          prev = None
          groups = [(b, g) for b in range(B) for g in range(G)]
          for bg, (b, g) in enumerate(groups):
                s0 = b * S + g * SG

                # ---- input streams for this group ----
                if rep == 0 and bg == 0:
                    xh0, xh1, cst = pre_xh0, pre_xh1, pre_cst
                elif mode == "nodmain" and hoist:
                    xh0, xh1, cst = hoist["t"]
                else:
                    xh0 = xpool.tile([128, 8, SG], BF16, tag="xt")
                    nc.sync.dma_start(
                        xh0[:],
                        xt[bg % (B * G)].rearrange(
                            "p (a s) -> p a s", s=SG)[:, 0:8, :])
                    xh1 = xpool.tile([128, 8, SG], BF16, tag="xt")
                    nc.sync.dma_start(
                        xh1[:],
                        xt[bg % (B * G)].rearrange(
                            "p (a s) -> p a s", s=SG)[:, 8:16, :])
                    cst = cspool.tile([128, 8, SG], BF16, tag="cs")
                    nc.sync.dma_start(
                        cst[:], cs[bg % (B * G)].rearrange(
                            "p (f s) -> p f s", s=SG))
                    if mode == "nodmain":
                        hoist["t"] = (xh0, xh1, cst)

                def xs(dk):
                    t = xh0 if dk < 8 else xh1
                    return t[:, dk % 8, :]

                if mode == "dmaonly":
                    if dummy_out is None:
                        dummy_out = outp.tile([128, D], BF16, tag="out")
                        nc.vector.memset(dummy_out[:], 0.0)
                    for st in range(4):
                        row = s0 + st * 128
                        nc.sync.dma_start(opart[row:row + 128, :],
                                          dummy_out[:])
                    continue

                # ---- QKV projections, token-major Q/K ----
                # psqk [128 tok, 512] = q_h0|q_h1|k_h0|k_h1 per 128-token
                # chunk; RMSNorm sums via ACT Square accum_out (no PE
                # ones-matmul), RoPE via shifted-view DVE muls (no PE rotate
                # matmul), and the head-major q^T/k^T layout the scores
                # matmuls need is produced by DMA xbar transposes (no PE).
                qkr = qpool.tile([128, 4, SG], BF16, tag="qkr")
                qr_tiles = {h: qkr[:, h, :] for h in range(HLOC)}
                for h in range(HLOC):
                    KrT[(b, h, g)] = qkr[:, 2 + h, :]

                for st in range(4):
                    xc = [xs(dk)[:, st * 128:(st + 1) * 128]
                          for dk in range(NDK)]
                    psqk = psA.tile([128, 512], F32, tag="a")
                    for dk in range(NDK):
                        nc.tensor.matmul(psqk[:], xc[dk], wqk_t[:, dk, :],
                                         start=(dk == 0),
                                         stop=(dk == NDK - 1))
                    psv = psA.tile([128, NL], F32, tag="a")
                    for dk in range(NDK):
                        nc.tensor.matmul(psv[:], xc[dk], wv_t[:, dk, :],
                                         start=(dk == 0),
                                         stop=(dk == NDK - 1))
                    vt = vpool.tile([128, NL], BF16, tag="v")
                    nc.scalar.copy(vt[:], psv[:])
                    Vt[(b, 4 * g + st)] = vt

                    # RMSNorm denominators: Square+accum per head block
                    scr = rpool.tile([128, 512], BF16, tag="scr")
                    rsums = rpool.tile([128, 4], F32, tag="rsums")
                    for c in range(4):
                        nc.scalar.activation(scr[:, c * 128:(c + 1) * 128],
                                             psqk[:, c * 128:(c + 1) * 128],
                                             AF.Square,
                                             accum_out=rsums[:, c:c + 1])
                    rstd4 = rpool.tile([128, 4], F32, tag="rstd4")
                    nc.scalar.activation(rstd4[:], rsums[:], AF.Ln,
                                         bias=eps_t[:, 0:1], scale=1.0 / HD)
                    nc.scalar.activation(rstd4[:], rstd4[:], AF.Exp,
                                         scale=-0.5)

                    # RoPE (rstd applied after; RoPE linear, rstd per-token)
                    ct = cst[:, 2 * st, :]       # cq|cq|ck|ck  [128, 512]
                    snt = cst[:, 2 * st + 1, :]  # signed+rolled sin pack
                    t1 = rpool.tile([128, 512], BF16, tag="t1")
                    nc.vector.tensor_mul(t1[:], psqk[:], ct)
                    t2 = rpool.tile([128, 512], BF16, tag="t2")
                    pv4 = psqk[:].rearrange("p (c h j) -> p c h j",
                                            c=4, j=64)
                    t4 = t2[:].rearrange("p (c h j) -> p c h j", c=4, j=64)
                    s4 = snt.rearrange("p (c h j) -> p c h j", c=4, j=64)
                    nc.vector.tensor_mul(t4[:, :, 0, :], pv4[:, :, 1, :],
                                         s4[:, :, 0, :])
                    nc.vector.tensor_mul(t4[:, :, 1, :], pv4[:, :, 0, :],
                                         s4[:, :, 1, :])
                    qk1 = rpool.tile([128, 512], BF16, tag="qk1")
                    nc.vector.tensor_add(qk1[:], t1[:], t2[:])
                    qk2 = rpool.tile([128, 512], BF16, tag="qk2")
                    for c in range(4):
                        nc.vector.tensor_scalar_mul(
                            qk2[:, c * 128:(c + 1) * 128],
                            qk1[:, c * 128:(c + 1) * 128],
                            rstd4[:, c:c + 1])
                    # head-major q^T/k^T via PE transpose (128 cyc each)
                    # + one strided DVE copy scattering all 4 blocks into
                    # the fused qkr tile
                    pst = psT.tile([128, 4, 128], BF16, tag="t")
                    for c in range(4):
                        nc.tensor.transpose(pst[:, c, :],
                                            qk2[:, c * 128:(c + 1) * 128],
                                            ident_t[:])
                    nc.vector.tensor_copy(
                        qkr[:, :, st * 128:(st + 1) * 128], pst[:])

                # Wo of group i-2 (osb ready since attention(i-2) in iter
                # i-1), then attention of group i-1: both run on PE after
                # proj(i), hiding the transpose/normalize tails.
                while pending_wo:
                    emit_wo(*pending_wo.pop(0))
                if prev is not None:
                    pb, pg, pqr, ps0 = prev
                    osbs = emit_attention(
                        pb, pg,  pqr,
                        {k[1:]: v for k, v in KrT.items() if k[0] == pb},
                        {k[1]: v for k, v in Vt.items() if k[0] == pb})
                    pending_wo.append((osbs, ps0))
                prev = (b, g, qr_tiles, s0)

          if mode != "dmaonly" and prev is not None:
                b, g, pqr, ps0 = prev
                osbs = emit_attention(
                    b, g, pqr,
                    {k[1:]: v for k, v in KrT.items() if k[0] == b},
                    {k[1]: v for k, v in Vt.items() if k[0] == b})
                pending_wo.append((osbs, ps0))
          while pending_wo:
                emit_wo(*pending_wo.pop(0))

    nc.compile()
    return nc


def _host_prep(hidden_states, cos, sin, Wq, Wk, Wv, Wo, q_scale, k_scale):
    import ml_dtypes
    f32 = np.float32
    bf16 = ml_dtypes.bfloat16
    hs = np.asarray(hidden_states, f32)
    cos = np.asarray(cos, f32)
    sin = np.asarray(sin, f32)
    qs = np.asarray(q_scale, f32)
    ks = np.asarray(k_scale, f32)

    def ptile(a2d, width):
        """[128*K, W] -> [128, K*W] partition-major contiguous pre-tiling."""
        k = a2d.shape[0] // 128
        return np.ascontiguousarray(
            a2d.reshape(k, 128, width).transpose(1, 0, 2).reshape(128, -1)
        ).astype(bf16)

    # xt: per (b,g) block of X^T, pre-tiled
    xt = np.stack([
        ptile(hs[b].T[:, g * SG:(g + 1) * SG], SG)
        for b in range(B) for g in range(G)
    ])   # [B*G, 128, 16*SG]

    # Token-major cos/sin packs. Signs + the head-dim roll of rotate_half
    # are folded into the sin tables; q_scale/k_scale/softmax SCALE folded
    # into both.
    qs_rot = np.roll(qs, -64)
    ks_rot = np.roll(ks, -64)
    sgn = np.concatenate([-np.ones(64, f32), np.ones(64, f32)])
    cq_full = [cos[b] * (qs * SCALE)[None, :] for b in range(B)]     # [S,HD]
    sq_full = [sin[b] * (qs_rot * SCALE * sgn)[None, :] for b in range(B)]
    ck_full = [cos[b] * ks[None, :] for b in range(B)]
    sk_full = [sin[b] * (ks_rot * sgn)[None, :] for b in range(B)]

    def cs_pack(b, g):
        out = np.empty((128, 8, 4 * HD), f32)
        for st in range(4):
            rows = slice(g * SG + st * 128, g * SG + (st + 1) * 128)
            out[:, 2 * st, :] = np.concatenate(
                [cq_full[b][rows], cq_full[b][rows],
                 ck_full[b][rows], ck_full[b][rows]], axis=1)
            out[:, 2 * st + 1, :] = np.concatenate(
                [sq_full[b][rows], sq_full[b][rows],
                 sk_full[b][rows], sk_full[b][rows]], axis=1)
        return out.reshape(128, -1)

    cs_all = np.ascontiguousarray(np.stack(
        [cs_pack(b, g) for b in range(B) for g in range(G)]).astype(bf16))

    ones = np.ones((128, 128), bf16)
    ident = np.eye(128, dtype=f32).astype(bf16)

    shared = {"xt": xt, "cs": cs_all, "ones_d": ones, "ident_d": ident}
    Wq = np.asarray(Wq, f32)
    Wk = np.asarray(Wk, f32)
    Wv = np.asarray(Wv, f32)
    Wo = np.asarray(Wo, f32)
    in_maps = []
    for c in range(NCORES):
        m = dict(shared)
        wqk2d = np.concatenate([Wq[:, c * NL:(c + 1) * NL],
                                Wk[:, c * NL:(c + 1) * NL]], axis=1)
        m["wqk"] = ptile(wqk2d, 4 * HD)
        m["wv"] = ptile(Wv[:, c * NL:(c + 1) * NL], NL)
        m["wo"] = ptile(Wo[c * NL:(c + 1) * NL, :], D)
        in_maps.append(m)
    return in_maps


def get_nc(repeat=1, mode="full"):
    key = ("nc", repeat, mode)
    if key not in _CACHE:
        _CACHE[key] = _build(repeat=repeat, mode=mode)
    return _CACHE[key]


def kernel(hidden_states, cos, sin, Wq, Wk, Wv, Wo, q_scale, k_scale):
    nc = get_nc()
    in_maps = _host_prep(hidden_states, cos, sin, Wq, Wk, Wv, Wo,
                         q_scale, k_scale)
    res = bass_utils.run_bass_kernel_spmd(nc, in_maps,
                                          core_ids=list(range(NCORES)))
    acc = np.zeros((B * S, D), np.float64)
    for r in res.results:
        acc += r["opart"].astype(np.float64)
    return np.ascontiguousarray(
        acc.reshape(B, S, D).astype(np.float32))

